# revision 13
# baseline (speedup 1.0000x reference)
"""HashEmbedder forward (2-level, res 1 & 3, F=2) as a Trainium2 Bass kernel.

Math: for each (level, coord, feature) the trilinear interp collapses to a 1-D
piecewise-linear function of one coordinate:
  level 0 (res=1, x in [0,1]):  out = k0 + q*x
  level 1 (res=3):              out = kap + gx*x + g1*clamp(x,1/3,2/3) + b2*relu(x-2/3)
Coefficients derive on the host from the tiny embedding tables (20 floats) and
ride along as a (128,37) per-partition constant tile; the per-point work (2M
points) is data-parallel across the 8 NeuronCores.

Layout: everything unit-stride on device (strided DVE ops measured ~10x slow).
Host pre-transposes x to (3, B/8) per core and un-transposes the (12, B/8)
blocked output. The clamp basis is one merged DVE op over all 3 coords; the
relu basis is one merged ACT op; per-column work is 2 ACT affines + 2 DVE
fused multiply-adds, all on contiguous (128, N) slices.
"""

import os

import numpy as np

import concourse.bass as bass  # noqa: F401  (engine types via nc.*)
import concourse.tile as tile
from concourse import bacc, mybir
from concourse.bass_utils import run_bass_kernel_spmd

B = 2_097_152
N_CORES = 8
PER_CORE = B // N_CORES  # 262144 points per core
PARTS = 128
ROWS = PER_CORE // PARTS  # 2048 points per partition per core
NPT = 1024  # points per partition per tile
NT = ROWS // NPT  # tiles per core
NCOEF = 37  # 6 per (d,f) + relu bias

# Exposed for test.py: BassKernelResults of the last kernel() call
LAST_RESULT = None


def _coeffs(emb_l0: np.ndarray, emb_l1: np.ndarray) -> np.ndarray:
    """(128, 37) f32: per (d, f): [q, k0, gx, kap, g1, b2]; col 36 = -2/3."""
    e0 = emb_l0.astype(np.float64)
    e1 = emb_l1.astype(np.float64)
    coef = np.zeros(NCOEF, np.float64)
    third = float(np.float32(1.0 / 3.0))
    for d in range(3):
        for f in range(2):
            c = (d * 2 + f) * 6
            coef[c + 0] = e0[d, 1, f] - e0[d, 0, f]  # q
            coef[c + 1] = e0[d, 0, f]  # k0
            V = e1[d, :, f]
            dk = V[1:] - V[:-1]
            c0, c1, c2 = 3.0 * dk[0], 3.0 * dk[1], 3.0 * dk[2]
            g1 = np.float64(np.float32(c1 - c0))
            coef[c + 2] = c0  # gx
            coef[c + 3] = V[0] - g1 * third  # kap
            coef[c + 4] = g1  # g1
            coef[c + 5] = c2 - c0  # b2
    coef[36] = -float(np.float32(2.0 / 3.0))
    return np.ascontiguousarray(
        np.broadcast_to(coef.astype(np.float32), (PARTS, NCOEF))
    )


def _build() -> bacc.Bacc:
    f32 = mybir.dt.float32
    Ident = mybir.ActivationFunctionType.Identity
    Relu = mybir.ActivationFunctionType.Relu
    Op = mybir.AluOpType
    THIRD = float(np.float32(1.0 / 3.0))
    TWO3 = float(np.float32(2.0 / 3.0))

    nc = bacc.Bacc()
    xt = nc.dram_tensor("xt", [3, PER_CORE], f32, kind="ExternalInput")
    coef = nc.dram_tensor("coef", [PARTS, NCOEF], f32, kind="ExternalInput")
    out = nc.dram_tensor("out", [12, PER_CORE], f32, kind="ExternalOutput")
    # blocked, partition-major views: every DMA run is 3/12 contiguous chunks
    xv = xt.rearrange("d (i p n) -> i p d n", p=PARTS, n=NPT)
    ov = out.rearrange("c (i p n) -> i p c n", p=PARTS, n=NPT)  # (NT,128,12,NPT)

    with tile.TileContext(nc) as tc:
        with tc.tile_pool(name="const", bufs=1) as cpool, tc.tile_pool(
            name="xin", bufs=2
        ) as xpool, tc.tile_pool(name="oout", bufs=2) as opool, tc.tile_pool(
            name="basis", bufs=2
        ) as bpool, tc.tile_pool(name="tmp", bufs=2) as tpool:
            ct = cpool.tile([PARTS, NCOEF], f32)

            def cc(c):
                return ct[:, c : c + 1]

            for i in range(NT):
                x3 = xpool.tile([PARTS, 3, NPT], f32, tag="x3")
                nc.sync.dma_start(out=x3[:], in_=xv[i])
                if i == 0:
                    # x load first: the clamp basis (immediates) can start
                    # the moment x lands; the tiny coef tile follows
                    nc.sync.dma_start(out=ct[:], in_=coef[:, :])
                # merged basis over all 3 coords in one op each
                s1B = bpool.tile([PARTS, 3, NPT], f32, tag="s1B")
                nc.vector.tensor_scalar(s1B[:], x3[:], THIRD, TWO3, Op.max, Op.min)
                r2B = bpool.tile([PARTS, 3, NPT], f32, tag="r2B")
                nc.scalar.activation(r2B[:], x3[:], Relu, bias=cc(36), scale=1.0)

                ot = opool.tile([PARTS, 12, NPT], f32, tag="ot")
                for d in range(3):
                    xd = x3[:, d, :]
                    for f in range(2):
                        cb = (d * 2 + f) * 6
                        # level-0 column: q*x + k0 (gpsimd, off the ACT/DVE
                        # critical path; contiguous so the Q7 loop streams)
                        nc.gpsimd.tensor_scalar(
                            ot[:, 2 * d + f, :],
                            xd,
                            cc(cb + 0),
                            cc(cb + 1),
                            Op.mult,
                            Op.add,
                        )
                        # level-1 column: ((gx*x + kap) + g1*s1) + b2*r2
                        u = tpool.tile([PARTS, NPT], f32, tag="u")
                        nc.scalar.activation(
                            u[:], xd, Ident, bias=cc(cb + 3), scale=cc(cb + 2)
                        )
                        z = tpool.tile([PARTS, NPT], f32, tag="z")
                        nc.vector.scalar_tensor_tensor(
                            out=z[:],
                            in0=s1B[:, d, :],
                            scalar=cc(cb + 4),
                            in1=u[:],
                            op0=Op.mult,
                            op1=Op.add,
                        )
                        nc.vector.scalar_tensor_tensor(
                            out=ot[:, 6 + 2 * d + f, :],
                            in0=r2B[:, d, :],
                            scalar=cc(cb + 5),
                            in1=z[:],
                            op0=Op.mult,
                            op1=Op.add,
                        )
                # quarter-split stores: transfers start as soon as 3 columns
                # are done and the final tail chunk is small
                for q in range(4):
                    nc.sync.dma_start(
                        out=ov[i][:, 3 * q : 3 * q + 3, :],
                        in_=ot[:, 3 * q : 3 * q + 3, :],
                    )
    nc.finalize()
    return nc


def kernel(x, box_min, box_max, emb_l0, emb_l1):
    global LAST_RESULT
    x = np.asarray(x, dtype=np.float32)
    assert x.shape == (B, 3), x.shape
    coef = _coeffs(
        np.asarray(emb_l0, dtype=np.float32), np.asarray(emb_l1, dtype=np.float32)
    )
    nc = _build()
    in_maps = [
        {
            "xt": np.ascontiguousarray(x[c * PER_CORE : (c + 1) * PER_CORE].T),
            "coef": coef,
        }
        for c in range(N_CORES)
    ]
    trace = bool(int(os.environ.get("KERNEL_TRACE", "0")))
    res = run_bass_kernel_spmd(nc, in_maps, list(range(N_CORES)), trace=trace)
    LAST_RESULT = res
    outp = np.empty((B, 12), np.float32)
    for c in range(N_CORES):
        outp[c * PER_CORE : (c + 1) * PER_CORE, :] = res.results[c]["out"].T
    return outp


# revision 14
# speedup vs baseline: 1.0957x; 1.0957x over previous
"""HashEmbedder forward (2-level, res 1 & 3, F=2) as a Trainium2 Bass kernel.

Math: for each (level, coord, feature) the trilinear interp collapses to a 1-D
piecewise-linear function of one coordinate:
  level 0 (res=1, x in [0,1]):  out = k0 + q*x
  level 1 (res=3):              out = kap + gx*x + g1*clamp(x,1/3,2/3) + b2*relu(x-2/3)
Coefficients derive on the host from the tiny embedding tables (20 floats) and
ride along as a (128,37) per-partition constant tile; the per-point work (2M
points) is data-parallel across the 8 NeuronCores.

Layout: everything unit-stride on device (strided DVE ops measured ~10x slow).
Host pre-transposes x to (3, B/8) per core and un-transposes the (12, B/8)
blocked output. The clamp basis is one merged DVE op over all 3 coords; the
relu basis is one merged ACT op; per-column work is 2 ACT affines + 2 DVE
fused multiply-adds, all on contiguous (128, N) slices.
"""

import os

import numpy as np

import concourse.bass as bass  # noqa: F401  (engine types via nc.*)
import concourse.tile as tile
from concourse import bacc, mybir
from concourse.bass_utils import run_bass_kernel_spmd

B = 2_097_152
N_CORES = 8
PER_CORE = B // N_CORES  # 262144 points per core
PARTS = 128
ROWS = PER_CORE // PARTS  # 2048 points per partition per core
NPT = 1024  # points per partition per tile
NT = ROWS // NPT  # tiles per core
NCOEF = 37  # 6 per (d,f) + relu bias

# Exposed for test.py: BassKernelResults of the last kernel() call
LAST_RESULT = None


def _coeffs(emb_l0: np.ndarray, emb_l1: np.ndarray) -> np.ndarray:
    """(128, 37) f32: per (d, f): [q, k0, gx, kap, g1, b2]; col 36 = -2/3."""
    e0 = emb_l0.astype(np.float64)
    e1 = emb_l1.astype(np.float64)
    coef = np.zeros(NCOEF, np.float64)
    third = float(np.float32(1.0 / 3.0))
    for d in range(3):
        for f in range(2):
            c = (d * 2 + f) * 6
            coef[c + 0] = e0[d, 1, f] - e0[d, 0, f]  # q
            coef[c + 1] = e0[d, 0, f]  # k0
            V = e1[d, :, f]
            dk = V[1:] - V[:-1]
            c0, c1, c2 = 3.0 * dk[0], 3.0 * dk[1], 3.0 * dk[2]
            g1 = np.float64(np.float32(c1 - c0))
            coef[c + 2] = c0  # gx
            coef[c + 3] = V[0] - g1 * third  # kap
            coef[c + 4] = g1  # g1
            coef[c + 5] = c2 - c0  # b2
    coef[36] = -float(np.float32(2.0 / 3.0))
    return np.ascontiguousarray(
        np.broadcast_to(coef.astype(np.float32), (PARTS, NCOEF))
    )


def _build() -> bacc.Bacc:
    f32 = mybir.dt.float32
    Ident = mybir.ActivationFunctionType.Identity
    Relu = mybir.ActivationFunctionType.Relu
    Op = mybir.AluOpType
    THIRD = float(np.float32(1.0 / 3.0))
    TWO3 = float(np.float32(2.0 / 3.0))

    nc = bacc.Bacc()
    xt = nc.dram_tensor("xt", [3, PER_CORE], f32, kind="ExternalInput")
    coef = nc.dram_tensor("coef", [PARTS, NCOEF], f32, kind="ExternalInput")
    out = nc.dram_tensor("out", [12, PER_CORE], f32, kind="ExternalOutput")
    # blocked, partition-major views: every DMA run is 3/12 contiguous chunks
    xv = xt.rearrange("d (i p n) -> i p d n", p=PARTS, n=NPT)
    ov = out.rearrange("c (i p n) -> i p c n", p=PARTS, n=NPT)  # (NT,128,12,NPT)

    with tile.TileContext(nc) as tc:
        with tc.tile_pool(name="const", bufs=1) as cpool, tc.tile_pool(
            name="xin", bufs=2
        ) as xpool, tc.tile_pool(name="oout", bufs=2) as opool, tc.tile_pool(
            name="basis", bufs=2
        ) as bpool, tc.tile_pool(name="tmp", bufs=2) as tpool:
            ct = cpool.tile([PARTS, NCOEF], f32)

            def cc(c):
                return ct[:, c : c + 1]

            for i in range(NT):
                x3 = xpool.tile([PARTS, 3, NPT], f32, tag="x3")
                nc.sync.dma_start(out=x3[:], in_=xv[i])
                if i == 0:
                    # x load first: the clamp basis (immediates) can start
                    # the moment x lands; the tiny coef tile follows
                    nc.sync.dma_start(out=ct[:], in_=coef[:, :])
                # merged basis over all 3 coords in one op each
                s1B = bpool.tile([PARTS, 3, NPT], f32, tag="s1B")
                nc.vector.tensor_scalar(s1B[:], x3[:], THIRD, TWO3, Op.max, Op.min)
                r2B = bpool.tile([PARTS, 3, NPT], f32, tag="r2B")
                nc.scalar.activation(r2B[:], x3[:], Relu, bias=cc(36), scale=1.0)

                ot = opool.tile([PARTS, 12, NPT], f32, tag="ot")
                for d in range(3):
                    xd = x3[:, d, :]
                    for f in range(2):
                        cb = (d * 2 + f) * 6
                        # level-0 column: q*x + k0
                        nc.scalar.activation(
                            ot[:, 2 * d + f, :],
                            xd,
                            Ident,
                            bias=cc(cb + 1),
                            scale=cc(cb + 0),
                        )
                        # level-1 column: ((gx*x + kap) + g1*s1) + b2*r2
                        u = tpool.tile([PARTS, NPT], f32, tag="u")
                        nc.scalar.activation(
                            u[:], xd, Ident, bias=cc(cb + 3), scale=cc(cb + 2)
                        )
                        z = tpool.tile([PARTS, NPT], f32, tag="z")
                        nc.vector.scalar_tensor_tensor(
                            out=z[:],
                            in0=s1B[:, d, :],
                            scalar=cc(cb + 4),
                            in1=u[:],
                            op0=Op.mult,
                            op1=Op.add,
                        )
                        nc.vector.scalar_tensor_tensor(
                            out=ot[:, 6 + 2 * d + f, :],
                            in0=r2B[:, d, :],
                            scalar=cc(cb + 5),
                            in1=z[:],
                            op0=Op.mult,
                            op1=Op.add,
                        )
                # quarter-split stores: transfers start as soon as 3 columns
                # are done and the final tail chunk is small
                for q in range(4):
                    nc.sync.dma_start(
                        out=ov[i][:, 3 * q : 3 * q + 3, :],
                        in_=ot[:, 3 * q : 3 * q + 3, :],
                    )
    nc.finalize()
    return nc


def kernel(x, box_min, box_max, emb_l0, emb_l1):
    global LAST_RESULT
    x = np.asarray(x, dtype=np.float32)
    assert x.shape == (B, 3), x.shape
    coef = _coeffs(
        np.asarray(emb_l0, dtype=np.float32), np.asarray(emb_l1, dtype=np.float32)
    )
    nc = _build()
    in_maps = [
        {
            "xt": np.ascontiguousarray(x[c * PER_CORE : (c + 1) * PER_CORE].T),
            "coef": coef,
        }
        for c in range(N_CORES)
    ]
    trace = bool(int(os.environ.get("KERNEL_TRACE", "0")))
    res = run_bass_kernel_spmd(nc, in_maps, list(range(N_CORES)), trace=trace)
    LAST_RESULT = res
    outp = np.empty((B, 12), np.float32)
    for c in range(N_CORES):
        outp[c * PER_CORE : (c + 1) * PER_CORE, :] = res.results[c]["out"].T
    return outp


# revision 15
# speedup vs baseline: 1.1093x; 1.0124x over previous
"""HashEmbedder forward (2-level, res 1 & 3, F=2) as a Trainium2 Bass kernel.

Math: for each (level, coord, feature) the trilinear interp collapses to a 1-D
piecewise-linear function of one coordinate:
  level 0 (res=1, x in [0,1]):  out = k0 + q*x
  level 1 (res=3):              out = kap + gx*x + g1*clamp(x,1/3,2/3) + b2*relu(x-2/3)
Coefficients derive on the host from the tiny embedding tables (20 floats) and
ride along as a (128,37) per-partition constant tile; the per-point work (2M
points) is data-parallel across the 8 NeuronCores.

Layout: everything unit-stride on device (strided DVE ops measured ~10x slow).
Host pre-transposes x to (3, B/8) per core and un-transposes the (12, B/8)
blocked output. The clamp basis is one merged DVE op over all 3 coords; the
relu basis is one merged ACT op; per-column work is 2 ACT affines + 2 DVE
fused multiply-adds, all on contiguous (128, N) slices.
"""

import os

import numpy as np

import concourse.bass as bass  # noqa: F401  (engine types via nc.*)
import concourse.tile as tile
from concourse import bacc, mybir
from concourse.bass_utils import run_bass_kernel_spmd

B = 2_097_152
N_CORES = 8
PER_CORE = B // N_CORES  # 262144 points per core
PARTS = 128
ROWS = PER_CORE // PARTS  # 2048 points per partition per core
NPT = 1024  # points per partition per tile
NT = ROWS // NPT  # tiles per core
NCOEF = 37  # 6 per (d,f) + relu bias

# Exposed for test.py: BassKernelResults of the last kernel() call
LAST_RESULT = None


def _coeffs(emb_l0: np.ndarray, emb_l1: np.ndarray) -> np.ndarray:
    """(128, 37) f32: per (d, f): [q, k0, gx, kap, g1, b2]; col 36 = -2/3."""
    e0 = emb_l0.astype(np.float64)
    e1 = emb_l1.astype(np.float64)
    coef = np.zeros(NCOEF, np.float64)
    third = float(np.float32(1.0 / 3.0))
    for d in range(3):
        for f in range(2):
            c = (d * 2 + f) * 6
            coef[c + 0] = e0[d, 1, f] - e0[d, 0, f]  # q
            coef[c + 1] = e0[d, 0, f]  # k0
            V = e1[d, :, f]
            dk = V[1:] - V[:-1]
            c0, c1, c2 = 3.0 * dk[0], 3.0 * dk[1], 3.0 * dk[2]
            g1 = np.float64(np.float32(c1 - c0))
            coef[c + 2] = c0  # gx
            coef[c + 3] = V[0] - g1 * third  # kap
            coef[c + 4] = g1  # g1
            coef[c + 5] = c2 - c0  # b2
    coef[36] = -float(np.float32(2.0 / 3.0))
    return np.ascontiguousarray(
        np.broadcast_to(coef.astype(np.float32), (PARTS, NCOEF))
    )


def _build() -> bacc.Bacc:
    f32 = mybir.dt.float32
    Ident = mybir.ActivationFunctionType.Identity
    Relu = mybir.ActivationFunctionType.Relu
    Op = mybir.AluOpType
    THIRD = float(np.float32(1.0 / 3.0))
    TWO3 = float(np.float32(2.0 / 3.0))

    nc = bacc.Bacc()
    xt = nc.dram_tensor("xt", [3, PER_CORE], f32, kind="ExternalInput")
    coef = nc.dram_tensor("coef", [PARTS, NCOEF], f32, kind="ExternalInput")
    out = nc.dram_tensor("out", [12, PER_CORE], f32, kind="ExternalOutput")
    # blocked, partition-major views: every DMA run is 3/12 contiguous chunks
    xv = xt.rearrange("d (i p n) -> i p d n", p=PARTS, n=NPT)
    ov = out.rearrange("c (i p n) -> i p c n", p=PARTS, n=NPT)  # (NT,128,12,NPT)

    with tile.TileContext(nc) as tc:
        with tc.tile_pool(name="const", bufs=1) as cpool, tc.tile_pool(
            name="xin", bufs=2
        ) as xpool, tc.tile_pool(name="oout", bufs=2) as opool, tc.tile_pool(
            name="basis", bufs=2
        ) as bpool, tc.tile_pool(name="tmp", bufs=2) as tpool:
            ct = cpool.tile([PARTS, NCOEF], f32)

            def cc(c):
                return ct[:, c : c + 1]

            for i in range(NT):
                x3 = xpool.tile([PARTS, 3, NPT], f32, tag="x3")
                nc.sync.dma_start(out=x3[:], in_=xv[i])
                if i == 0:
                    # x load first: the clamp basis (immediates) can start
                    # the moment x lands; the tiny coef tile follows
                    nc.sync.dma_start(out=ct[:], in_=coef[:, :])
                # merged basis over all 3 coords in one op each
                s1B = bpool.tile([PARTS, 3, NPT], f32, tag="s1B")
                nc.vector.tensor_scalar(s1B[:], x3[:], THIRD, TWO3, Op.max, Op.min)
                r2B = bpool.tile([PARTS, 3, NPT], f32, tag="r2B")
                nc.scalar.activation(r2B[:], x3[:], Relu, bias=cc(36), scale=1.0)

                ot = opool.tile([PARTS, 12, NPT], f32, tag="ot")
                for d in range(3):
                    xd = x3[:, d, :]
                    for f in range(2):
                        cb = (d * 2 + f) * 6
                        # level-0 column: q*x + k0 (d=0 pair on DVE to balance
                        # engines; DVE 1-input TS can hit the 2x f32 mode)
                        if d == 0:
                            nc.vector.tensor_scalar(
                                ot[:, 2 * d + f, :],
                                xd,
                                cc(cb + 0),
                                cc(cb + 1),
                                Op.mult,
                                Op.add,
                            )
                        else:
                            nc.scalar.activation(
                                ot[:, 2 * d + f, :],
                                xd,
                                Ident,
                                bias=cc(cb + 1),
                                scale=cc(cb + 0),
                            )
                        # level-1 column: ((gx*x + kap) + g1*s1) + b2*r2
                        u = tpool.tile([PARTS, NPT], f32, tag="u")
                        nc.scalar.activation(
                            u[:], xd, Ident, bias=cc(cb + 3), scale=cc(cb + 2)
                        )
                        z = tpool.tile([PARTS, NPT], f32, tag="z")
                        nc.vector.scalar_tensor_tensor(
                            out=z[:],
                            in0=s1B[:, d, :],
                            scalar=cc(cb + 4),
                            in1=u[:],
                            op0=Op.mult,
                            op1=Op.add,
                        )
                        nc.vector.scalar_tensor_tensor(
                            out=ot[:, 6 + 2 * d + f, :],
                            in0=r2B[:, d, :],
                            scalar=cc(cb + 5),
                            in1=z[:],
                            op0=Op.mult,
                            op1=Op.add,
                        )
                # quarter-split stores: transfers start as soon as 3 columns
                # are done and the final tail chunk is small
                for q in range(4):
                    nc.sync.dma_start(
                        out=ov[i][:, 3 * q : 3 * q + 3, :],
                        in_=ot[:, 3 * q : 3 * q + 3, :],
                    )
    nc.finalize()
    return nc


def kernel(x, box_min, box_max, emb_l0, emb_l1):
    global LAST_RESULT
    x = np.asarray(x, dtype=np.float32)
    assert x.shape == (B, 3), x.shape
    coef = _coeffs(
        np.asarray(emb_l0, dtype=np.float32), np.asarray(emb_l1, dtype=np.float32)
    )
    nc = _build()
    in_maps = [
        {
            "xt": np.ascontiguousarray(x[c * PER_CORE : (c + 1) * PER_CORE].T),
            "coef": coef,
        }
        for c in range(N_CORES)
    ]
    trace = bool(int(os.environ.get("KERNEL_TRACE", "0")))
    res = run_bass_kernel_spmd(nc, in_maps, list(range(N_CORES)), trace=trace)
    LAST_RESULT = res
    outp = np.empty((B, 12), np.float32)
    for c in range(N_CORES):
        outp[c * PER_CORE : (c + 1) * PER_CORE, :] = res.results[c]["out"].T
    return outp
